# revision 7
# baseline (speedup 1.0000x reference)
"""Multi-head attention forward on 8 Trainium2 NeuronCores.

Problem (hardcoded): B=4, N=M=2048, D=1024, H=16, HS=64, OUT=1024, fp32.

Sharding: 8 cores = 4 batches x 2 head-groups of 8 heads. Each core
computes a partial output [2048, 1024] = sum over its 8 heads of
softmax((X_q Wq_h)(X_k Wk_h)^T / 8) (X_v Wv_h) Wo_h.  Host sums the two
head-group partials per batch and adds the projection bias.

v3 design:
  * All-fp16 PE data path (x, weights, qt/kt, V, exp, ctxn): single-pass
    matmuls with FWL-eligible 128-col stationaries, fp16 transposes.
  * All weight DMAs prefetched at kernel start (ACT queue is idle then);
    no mid-stream weight-load stalls.
  * Phase 1 order: Q transposes (DMA-paced, kept resident), K stream,
    V stream, Q-projection of pair 0.  Q-projections of pairs 1-3 are
    interleaved into the PE slack of pair 0's (ACT-paced) attention.
  * Logits for the two heads of a pair run CONCURRENTLY as row-tiled
    K=64 matmuls (tile_position (0,0)/(64,0)); exp once per step over
    both heads' logits [128, 2x512] on ScalarE (the attention pacer).
  * ctx odd-head partition placement via SBUF->SBUF partition-hop DMA
    (idle DMA engines) instead of PE shift matmuls.
  * Per-(pair, n-quarter) deferred normalization; the output projection
    is a dedicated phase with a 4-deep PSUM ring, ordered so its first
    12 n-tiles never wait on the final quarter's normalization.
"""

import os
import sys

import numpy as np

for _p in ("/opt/trn_rl_repo",):
    if _p not in sys.path and os.path.isdir(_p):
        sys.path.insert(0, _p)

B, N, M, D = 4, 2048, 2048, 1024
H, HS, OUT = 16, 64, 1024
HL = 8          # heads per core
P = 128
NPAIR = HL // 2  # head pairs per core
DT = D // P      # 8 d-tiles
NT = N // P      # 16 n-tiles
MT = M // P      # 16 m-tiles
NQ = 512         # n-quarter width per attention block
PIPE = 3         # ctx trails logits by PIPE m-tiles


def build_mha(tc, ins, out_ap):
    import concourse.bass as bass
    from concourse import mybir

    nc = tc.nc
    f32 = mybir.dt.float32
    f16 = mybir.dt.float16

    xq, xk, xv = ins["xq"], ins["xk"], ins["xv"]
    wq, wk, wv, wo = ins["wq"], ins["wk"], ins["wv"], ins["wo"]

    import contextlib

    with contextlib.ExitStack() as ctx:
        # ---- constant tiles ----
        const = ctx.enter_context(tc.tile_pool(name="const", bufs=1))
        identity = const.tile([P, P], f32)
        from concourse.masks import make_identity
        make_identity(nc, identity)
        identity_h = const.tile([P, P], f16)
        nc.vector.tensor_copy(identity_h[:], identity[:])
        ones_f32 = const.tile([P, HL], f32)
        nc.vector.memset(ones_f32[:], 1.0)
        ones_h = const.tile([P, HL], f16)
        nc.vector.tensor_copy(ones_h[:], ones_f32[:])
        # head-select mask: hmask2[0:2, s, :] is 1 on partition s, else 0.
        hmask2 = const.tile([2, 2, 64], f32)
        nc.gpsimd.memset(hmask2[:], 0.0)
        nc.gpsimd.affine_select(
            out=hmask2[:],
            in_=hmask2[:],
            compare_op=mybir.AluOpType.not_equal,
            fill=1.0,
            base=0,
            pattern=[[-1, 2], [0, 64]],
            channel_multiplier=1,
        )
        hmask2_h = const.tile([2, 2, 64], f16)
        nc.vector.tensor_copy(hmask2_h[:], hmask2[:])

        # ---- persistent activations ----
        act_pool = ctx.enter_context(tc.tile_pool(name="acts", bufs=1))
        qt = [act_pool.tile([P, N], f16, name=f"qt{p}", tag=f"qt{p}") for p in range(NPAIR)]
        kt = [act_pool.tile([P, M], f16, name=f"kt{p}", tag=f"kt{p}") for p in range(NPAIR)]
        v_all = [act_pool.tile([P, HL, 66], f16, name=f"v{t}", tag=f"v{t}") for t in range(MT)]
        wo_sb = act_pool.tile([P, NPAIR, OUT], f16, name="wo_sb", tag="wo_sb")
        ctxn = [act_pool.tile([P, N], f16, name=f"ctxn{p}", tag=f"ctxn{p}")
                for p in range(NPAIR)]
        # transposed xq, kept resident so Q-projections can run during
        # attention; wq weights likewise persistent.
        xqt = act_pool.tile([P, DT, N], f16, name="xqt", tag="xqt")
        wq_sb = act_pool.tile([P, DT, HL, HS], f16, name="wq_sb", tag="wq_sb")

        # ---- phase 0: all weight loads up-front ----
        wkv_pool = ctx.enter_context(tc.tile_pool(name="wkv", bufs=1))
        with tc.tile_pool(name="wstage", bufs=4) as wstage_pool:
            def load_w(w_dram, w_sb):
                # w [8, 1024, 64] -> SBUF [128(d in tile), dt, h, 64] (f16)
                for dt_i in range(DT):
                    w_stage = wstage_pool.tile([P, HL, HS], f32, name="w_stage", tag="wst")
                    nc.scalar.dma_start(
                        w_stage[:],
                        w_dram[:, dt_i * P:(dt_i + 1) * P, :].rearrange("h p o -> p h o"))
                    nc.vector.tensor_copy(w_sb[:, dt_i, :, :], w_stage[:])

            wk_sb = wkv_pool.tile([P, DT, HL, HS], f16, name="wk_sb", tag="wk_sb")
            wv_sb = wkv_pool.tile([P, DT, HL, HS], f16, name="wv_sb", tag="wv_sb")
            load_w(wk, wk_sb)
            load_w(wv, wv_sb)
            load_w(wq, wq_sb)
            wo_stage = wstage_pool.tile(
                [P, NPAIR, OUT], f32, name="wo_stage", tag="wost")
            for s in range(2):
                nc.scalar.dma_start(
                    wo_stage[s * 64:(s + 1) * 64, :, :],
                    wo[s::2, :, :].rearrange("pp o d -> o pp d"))
            nc.vector.tensor_copy(wo_sb[:], wo_stage[:])

        # ---- phase 1: streamed load, transpose, project ----
        with tc.tile_pool(name="xt", bufs=2) as xt_pool, \
             tc.tile_pool(name="x_stream", bufs=4) as x_stream, \
             tc.tile_pool(name="tp_psum", bufs=4, space="PSUM") as tp_psum, \
             tc.tile_pool(name="proj_psum", bufs=4, space="PSUM") as proj_psum:

            def load_transpose_chunk(x_dram, dst_ap, n_tile):
                # one 128-row chunk: DMA, fp16 round (ScalarE), 8 PE
                # transposes in 2 groups of 4, wide DVE evictions.
                x_t = x_stream.tile([P, D], f32, name="x_t", tag="x_t")
                nc.sync.dma_start(
                    x_t[:], x_dram[n_tile * P:(n_tile + 1) * P, :])
                x_tr = x_stream.tile([P, D], f16, name="x_tr", tag="x_tr")
                nc.scalar.copy(x_tr[:], x_t[:])
                for g in range(2):
                    tp = tp_psum.tile([P, 4, P], f16, name="tp", tag="tp")
                    for q in range(4):
                        dt_i = 4 * g + q
                        nc.tensor.transpose(
                            tp[:, q, :], x_tr[:, dt_i * P:(dt_i + 1) * P],
                            identity_h[:])
                    nc.vector.tensor_copy(
                        dst_ap(slice(4 * g, 4 * g + 4), n_tile), tp[:])

            def stream_input(x_dram, w_sb, kind):
                # software pipeline: transposes of chunk c+1 interleave with
                # the projection matmuls of chunk c.
                NC = NT // 4
                xt_tiles = {}

                def do_transpose(c, j):
                    if j == 0:
                        xt_tiles[c] = xt_pool.tile(
                            [P, DT, 512], f16, name="xt_c", tag="xt_c")
                    xt_c = xt_tiles[c]
                    load_transpose_chunk(
                        x_dram,
                        lambda dts, nt, _c=c: xt_c[:, dts, (nt - 4 * _c) * P:(nt - 4 * _c + 1) * P],
                        4 * c + j)

                def do_proj(c, j):
                    xt_c = xt_tiles[c]
                    if kind == "v":
                        t = 4 * c + j
                        ps = proj_psum.tile([P, 512], f32, name="pp", tag="pp")
                        for dt_i in range(DT):
                            nc.tensor.matmul(
                                ps[:],
                                xt_c[:, dt_i, j * P:(j + 1) * P],
                                w_sb[:, dt_i, :, :],
                                start=(dt_i == 0), stop=(dt_i == DT - 1),
                            )
                        nc.vector.tensor_copy(
                            v_all[t][:, :, 0:64],
                            ps[:].rearrange("p (h o) -> p h o", h=HL))
                        nc.vector.tensor_copy(
                            v_all[t][:, :, 64:65],
                            ones_h[:, 0:HL].rearrange("p (h one) -> p h one", one=1))
                    else:
                        p = j
                        ps = proj_psum.tile([P, 512], f32, name="pp", tag="pp")
                        for dt_i in range(DT):
                            nc.tensor.matmul(
                                ps[:],
                                w_sb[:, dt_i, 2 * p:2 * p + 2, :],
                                xt_c[:, dt_i, :],
                                start=(dt_i == 0), stop=(dt_i == DT - 1),
                            )
                        nc.vector.tensor_copy(
                            kt[p][:, c * 512:(c + 1) * 512], ps[:])

                for j in range(4):
                    do_transpose(0, j)
                for c in range(NC):
                    for j in range(4):
                        if c + 1 < NC:
                            do_transpose(c + 1, j)
                        do_proj(c, j)
                    del xt_tiles[c]

            # Q transposes first (DMA-paced; xqt kept resident)
            for n_tile in range(NT):
                load_transpose_chunk(
                    xq,
                    lambda dts, nt: xqt[:, dts, nt * P:(nt + 1) * P],
                    n_tile)
            # K and V streams (PE-dense)
            stream_input(xk, wk_sb, "k")
            stream_input(xv, wv_sb, "v")

        # ---- phase 2: Q proj of pair 0 + attention ----
        # q-projection of one (pair, c) chunk: 8 matmuls + 1 DVE evict
        def emit_qproj_chunk(qproj_psum, p, c):
            ps = qproj_psum.tile([P, 512], f32, name="qp", tag="qp")
            for dt_i in range(DT):
                nc.tensor.matmul(
                    ps[:],
                    wq_sb[:, dt_i, 2 * p:2 * p + 2, :],
                    xqt[:, dt_i, c * 512:(c + 1) * 512],
                    start=(dt_i == 0), stop=(dt_i == DT - 1),
                )
            nc.vector.tensor_copy(qt[p][:, c * 512:(c + 1) * 512], ps[:])

        # PSUM budget (8 banks): lg ring 2 x 2 banks + cps0/cps1 1 bank each
        # + norm broadcast 1 + q-proj 1 = 8.
        with tc.tile_pool(name="lgs", bufs=PIPE + 2) as lgs_pool, \
             tc.tile_pool(name="sstage", bufs=3) as sstage_pool, \
             tc.tile_pool(name="sums", bufs=2) as sums_pool, \
             tc.tile_pool(name="lg_psum", bufs=2, space="PSUM") as lg_psum, \
             tc.tile_pool(name="ctx_psum", bufs=1, space="PSUM") as ctx_psum, \
             tc.tile_pool(name="nrm_psum", bufs=1, space="PSUM") as nrm_psum, \
             tc.tile_pool(name="qproj_psum", bufs=1, space="PSUM") as qproj_psum:

            for c in range(4):
                emit_qproj_chunk(qproj_psum, 0, c)

            sums_pair = {}
            sums_h_pair = {}

            def get_sums(p):
                if p not in sums_pair:
                    sums_pair[p] = sums_pool.tile(
                        [2, N], f32, name=f"sums{p}", tag="sums")
                    sums_h_pair[p] = sums_pool.tile(
                        [2, N], f16, name=f"sumsh{p}", tag="sumsh")
                return sums_pair[p]

            blocks = [(p, nq) for p in range(NPAIR) for nq in range(N // NQ)]
            steps = [(bi, t) for bi in range(len(blocks)) for t in range(MT)]
            cps_map = {}
            ets = {}

            def emit_logits(bi, t):
                p, nq = blocks[bi]
                n0 = nq * NQ
                lg = lg_psum.tile([P, 2, NQ], f32, name="lg", tag="lg")
                nc.tensor.matmul(
                    lg[:, 0, :],
                    kt[p][0:64, t * P:(t + 1) * P],
                    qt[p][0:64, n0:n0 + NQ],
                    start=True, stop=True,
                )
                nc.tensor.matmul(
                    lg[:, 1, :],
                    kt[p][64:128, t * P:(t + 1) * P],
                    qt[p][64:128, n0:n0 + NQ],
                    start=True, stop=True,
                )
                et = lgs_pool.tile([P, 2, NQ], f16, name="et", tag="et")
                nc.scalar.activation(
                    et[:, :, :], lg[:, :, :],
                    mybir.ActivationFunctionType.Exp, scale=0.125)
                ets[(bi, t)] = et

            def emit_evict(bi):
                # stage denominator rows (DVE copy + partition-hop DMA);
                # evict ctx rows UN-normalized; head 1 hops partitions
                # 0:64 -> 64:128 via SBUF->SBUF DMA.
                p, nq = blocks[bi]
                n0 = nq * NQ
                cps0, cps1 = cps_map.pop(bi)
                sums = get_sums(p)
                sstage = sstage_pool.tile([P, 2, NQ], f32, name="sst", tag="sst")
                nc.vector.tensor_copy(sstage[64:65, 0, :], cps0[64:65, :])
                nc.vector.tensor_copy(sstage[64:65, 1, :], cps1[64:65, :])
                nc.sync.dma_start(sums[0:1, n0:n0 + NQ], sstage[64:65, 0, :])
                nc.sync.dma_start(sums[1:2, n0:n0 + NQ], sstage[64:65, 1, :])
                nc.vector.tensor_copy(ctxn[p][0:64, n0:n0 + NQ], cps0[0:64, :])
                tmp = sstage_pool.tile([64, NQ], f16, name="ctmp", tag="ctmp")
                nc.vector.tensor_copy(tmp[:], cps1[0:64, :])
                nc.sync.dma_start(ctxn[p][64:128, n0:n0 + NQ], tmp[:])

            def emit_norm(bi):
                p, nq = blocks[bi]
                n0 = nq * NQ
                sums = sums_pair[p]
                sums_h = sums_h_pair[p]
                nc.vector.reciprocal_approx_fast(
                    sums[:, n0:n0 + NQ], sums[:, n0:n0 + NQ])
                nc.vector.tensor_copy(sums_h[:, n0:n0 + NQ], sums[:, n0:n0 + NQ])
                bc = nrm_psum.tile([P, NQ], f32, name="bc", tag="bc")
                for s in range(2):
                    nc.tensor.matmul(
                        bc[s * 64:(s + 1) * 64, :],
                        hmask2_h[:, s, :],
                        sums_h[:, n0:n0 + NQ],
                        start=True, stop=True,
                    )
                nc.vector.tensor_mul(
                    ctxn[p][:, n0:n0 + NQ], ctxn[p][:, n0:n0 + NQ], bc[:])

            def emit_ctx(bi, t):
                p, nq = blocks[bi]
                if t == 0:
                    cps_map[bi] = (
                        ctx_psum.tile([P, NQ], f32, name="cps0", tag="cps0"),
                        ctx_psum.tile([P, NQ], f32, name="cps1", tag="cps1"),
                    )
                cps0, cps1 = cps_map[bi]
                et = ets.pop((bi, t))
                nc.tensor.matmul(
                    cps0[0:65, :],
                    v_all[t][:, 2 * p, 0:65],
                    et[:, 0, :],
                    start=(t == 0), stop=(t == MT - 1),
                )
                nc.tensor.matmul(
                    cps1[0:65, :],
                    v_all[t][:, 2 * p + 1, 0:65],
                    et[:, 1, :],
                    start=(t == 0), stop=(t == MT - 1),
                )
                if t == MT - 1:
                    emit_evict(bi)

            # deferred Q-proj chunks injected into pair-0 attention slack:
            # (pair, c) for pairs 1-3, one chunk every 4 steps.
            qchunks = [(p, c) for p in range(1, NPAIR) for c in range(4)]

            pending = []
            normed = set()
            for i, (bi, t) in enumerate(steps):
                emit_logits(bi, t)
                if i % 4 == 1 and i // 4 < len(qchunks):
                    emit_qproj_chunk(qproj_psum, *qchunks[i // 4])
                if i >= PIPE:
                    cbi, ct = steps[i - PIPE]
                    emit_ctx(cbi, ct)
                    if ct == MT - 1:
                        pending.append((i + 4, cbi))
                while pending and pending[0][0] <= i:
                    _, nbi = pending.pop(0)
                    emit_norm(nbi)
                    normed.add(nbi)
            for i in range(len(steps) - PIPE, len(steps)):
                emit_ctx(*steps[i])
            for _, nbi in pending:
                emit_norm(nbi)
                normed.add(nbi)
            for bi in range(len(blocks)):
                if bi not in normed:
                    emit_norm(bi)

        # ---- phase 3: output projection (dedicated pools, deep ring) ----
        with tc.tile_pool(name="out_psum", bufs=4, space="PSUM") as out_psum, \
             tc.tile_pool(name="out_sb", bufs=3) as out_pool:
            for tile_n in range(NT):
                ot = out_pool.tile([P, OUT], f32, name="ot", tag="ot")
                for c in range(OUT // 512):
                    ops = out_psum.tile([P, 512], f32, name="ops", tag="ops")
                    for p in range(NPAIR):
                        nc.tensor.matmul(
                            ops[:],
                            ctxn[p][:, tile_n * P:(tile_n + 1) * P],
                            wo_sb[:, p, c * 512:(c + 1) * 512],
                            start=(p == 0), stop=(p == NPAIR - 1),
                        )
                    if (2 * tile_n + c) % 2 == 0:
                        nc.scalar.copy(ot[:, c * 512:(c + 1) * 512], ops[:])
                    else:
                        nc.vector.tensor_copy(ot[:, c * 512:(c + 1) * 512], ops[:])
                nc.sync.dma_start(out_ap[tile_n * P:(tile_n + 1) * P, :], ot[:])


def build_nc():
    import concourse.bacc as bacc
    import concourse.tile as tile
    from concourse import mybir

    nc = bacc.Bacc("TRN2", target_bir_lowering=False, debug=False)
    f32 = mybir.dt.float32
    ins = {
        "xq": nc.dram_tensor("xq", (N, D), f32, kind="ExternalInput").ap(),
        "xk": nc.dram_tensor("xk", (M, D), f32, kind="ExternalInput").ap(),
        "xv": nc.dram_tensor("xv", (M, D), f32, kind="ExternalInput").ap(),
        "wq": nc.dram_tensor("wq", (HL, D, HS), f32, kind="ExternalInput").ap(),
        "wk": nc.dram_tensor("wk", (HL, D, HS), f32, kind="ExternalInput").ap(),
        "wv": nc.dram_tensor("wv", (HL, D, HS), f32, kind="ExternalInput").ap(),
        "wo": nc.dram_tensor("wo", (HL, HS, OUT), f32, kind="ExternalInput").ap(),
    }
    out_ap = nc.dram_tensor("out", (N, OUT), f32, kind="ExternalOutput").ap()
    with tile.TileContext(nc) as tc:
        build_mha(tc, ins, out_ap)
    nc.compile()
    return nc


def make_in_maps(inputs):
    q = np.ascontiguousarray(np.asarray(inputs["query"], dtype=np.float32))
    k = np.ascontiguousarray(np.asarray(inputs["key"], dtype=np.float32))
    v = np.ascontiguousarray(np.asarray(inputs["value"], dtype=np.float32))
    wq = np.asarray(inputs["query_kernel"], dtype=np.float32)
    wk = np.asarray(inputs["key_kernel"], dtype=np.float32)
    wv = np.asarray(inputs["value_kernel"], dtype=np.float32)
    wo = np.asarray(inputs["projection_kernel"], dtype=np.float32)
    in_maps = []
    for c in range(8):
        b, hg = divmod(c, 2)
        hs = slice(hg * HL, (hg + 1) * HL)
        in_maps.append({
            "xq": q[b], "xk": k[b], "xv": v[b],
            "wq": np.ascontiguousarray(wq[hs]),
            "wk": np.ascontiguousarray(wk[hs]),
            "wv": np.ascontiguousarray(wv[hs]),
            "wo": np.ascontiguousarray(wo[hs]),
        })
    return in_maps


def combine(results, bias):
    out = np.empty((B, N, OUT), dtype=np.float32)
    for b in range(B):
        out[b] = results[2 * b]["out"] + results[2 * b + 1]["out"]
    out += np.asarray(bias, dtype=np.float32)[None, None, :]
    return out


_NC_CACHE = None


def _enable_ldw_opt():
    # kept as a no-op hook for test.py compatibility
    return


def kernel(**inputs):
    global _NC_CACHE
    from concourse import bass_utils
    _enable_ldw_opt()

    if _NC_CACHE is None:
        _NC_CACHE = build_nc()
    nc = _NC_CACHE
    in_maps = make_in_maps(inputs)
    res = bass_utils.run_bass_kernel_spmd(nc, in_maps, core_ids=list(range(8)))
    return combine(res.results, inputs["projection_bias"])
